# revision 17
# baseline (speedup 1.0000x reference)
"""Baichuan paged-attention layer on 8 trn2 cores, tensor-parallel over heads.

Per core c: heads 4c..4c+3 (W=512 of HID). The QKV projection and o_proj
run as 3-term residual-fp8 DoubleRow matmuls (x~=s*(x8+r8), terms
x8*w8 + x8*rw8 + rx8*w8 accumulate in one PSUM group at a shared
power-of-2 scale) — 256-deep contraction per PE instruction at fp8 rate.
Attention (scores/PV, contraction D=128) stays bf16/f32r. Softmax
denominators run on gpsimd partition_all_reduce instead of PE
ones-matmuls. History KV and rotated q/k live in bf16; output partials
are written f16 and summed on host. All scales are baked into activation
immediates (exp scale, psum->sbuf copy scales), so no extra scaling ops.
"""
import sys

sys.path.insert(0, "/opt/trn_rl_repo")
import numpy as np

H = 32; D = 128; HID = 4096; BS = 64; NBLOCKS = 128
B = 4; QLEN = 512; MAXBLK = 24; ROPE_BASE = 10000.0
T = B * QLEN; NCORES = 8; HC = H // NCORES; W = HC * D  # 4 heads, 512 wide
NEG = -1.0e30
SCALE = 1.0 / float(np.sqrt(D))
S_A = 32.0 / 224.0  # fixed attn-out quantization scale (|attn| << 32)

_cache = {}
last_results = None  # BassKernelResults of the most recent run (for test.py)

BUFS = dict(cs=2, hid=4, wq=3, wv=8, qkr=12, qs=2, ropet=2, vsb=6,
            kh=5, vh=2, exp=3, smol=2, stg=3, attn=5, ar=5, wo=4)


def _round128(x):
    return (x + 127) // 128 * 128


def _pow2_scale(maxval):
    """power-of-2 scale s with maxval/s <= 224 (e4m3 max is 240)"""
    return float(2.0 ** np.ceil(np.log2(maxval / 224.0)))


def _build(hist, scales, reps=1):
    import concourse.tile as tile
    from concourse import bacc, mybir, bass_isa

    F32 = mybir.dt.float32
    F32R = mybir.dt.float32r
    BF16 = mybir.dt.bfloat16
    F16 = mybir.dt.float16
    F8 = mybir.dt.float8e4
    DR = mybir.MatmulPerfMode.DoubleRow

    s_h, s_wqk, s_wv, s_wo = scales
    # exp immediates: history scores are q_raw * k_true; new are q_raw * k_raw
    exp_hist = SCALE * (s_h * s_wqk)
    exp_new = SCALE * (s_h * s_wqk) ** 2
    v_copy = s_h * s_wv          # v psum raw -> true values
    o_copy = S_A * s_wo          # o_proj psum raw -> true outputs

    hv = [_round128(h) for h in hist]
    SH = [x // 128 for x in hv]

    nc = bacc.Bacc("TRN2", target_bir_lowering=False, debug=False,
                   num_devices=NCORES)
    # hidden pair: [HID, T] fp8 each
    ha_d = nc.dram_tensor("ha", [HID, T], F8, kind="ExternalInput")
    hr_d = nc.dram_tensor("hr", [HID, T], F8, kind="ExternalInput")
    # wqk pairs: [128, 8*32*128] fp8 (p-major: partition, rt, slab, col)
    wqa_d = nc.dram_tensor("wqa", [128, 8 * 32 * 128], F8, kind="ExternalInput")
    wqr_d = nc.dram_tensor("wqr", [128, 8 * 32 * 128], F8, kind="ExternalInput")
    # wv pairs: [HID, W] fp8
    wva_d = nc.dram_tensor("wva", [HID, W], F8, kind="ExternalInput")
    wvr_d = nc.dram_tensor("wvr", [HID, W], F8, kind="ExternalInput")
    # wo pairs: [W, HID] fp8
    woa_d = nc.dram_tensor("woa", [W, HID], F8, kind="ExternalInput")
    wor_d = nc.dram_tensor("wor", [W, HID], F8, kind="ExternalInput")
    kh_d = [nc.dram_tensor(f"khT{b}", [W, hv[b]], BF16, kind="ExternalInput")
            if hv[b] else None for b in range(B)]
    vh_d = [nc.dram_tensor(f"vh{b}", [hv[b], W], BF16, kind="ExternalInput")
            if hv[b] else None for b in range(B)]
    out_d = nc.dram_tensor("out", [T, HID], F16, kind="ExternalOutput")

    # host-built tables baked into the NEFF
    inv = 1.0 / (ROPE_BASE ** (np.arange(0, D, 2) / D))
    pos = np.concatenate([h + np.arange(QLEN) for h in hist]).astype(np.float64)
    ang = np.concatenate([inv, inv])[:, None] * pos[None, :]
    cos_d = nc.inline_tensor(np.cos(ang).astype(np.float32), name="cosT")
    sin_d = nc.inline_tensor(np.sin(ang).astype(np.float32), name="sinT")

    mask_np = np.where(
        np.arange(128)[:, None] <= np.arange(896)[None, :] - 384,
        0.0, NEG).astype(np.float32)
    mask_d = nc.inline_tensor(mask_np, name="maskS")

    pad_np = np.zeros((128, B), np.float32)
    for b in range(B):
        if hv[b]:
            pad_np[:, b] = np.where(hv[b] - 128 + np.arange(128) >= hist[b],
                                    NEG, 0.0)
    pad_d = nc.inline_tensor(pad_np, name="padc")

    Pm = np.zeros((128, 128), np.float32)
    for d in range(64):
        Pm[d, d + 64] = -1.0
        Pm[d + 64, d] = 1.0
    pt_d = nc.inline_tensor(np.ascontiguousarray(Pm.T), name="permT")

    with tile.TileContext(nc) as tc:
        with tc.tile_pool(name="const", bufs=1) as cpool, \
             tc.tile_pool(name="attn", bufs=BUFS["attn"]) as apool, \
             tc.tile_pool(name="psum", bufs=8, space="PSUM") as pspool:
            mask_t = cpool.tile([128, 896], F32, tag="mask")
            pad_t = cpool.tile([128, B], F32, tag="pad")
            pt_t = cpool.tile([128, 128], F32R, tag="pt")
            const_dmas = [False]

            def emit_const_dmas():
                if const_dmas[0]:
                    return
                const_dmas[0] = True
                nc.sync.dma_start(pt_t[:], pt_d[:].bitcast(F32R))
                nc.sync.dma_start(mask_t[:], mask_d[:])
                nc.sync.dma_start(pad_t[:], pad_d[:])

            for rep in range(reps):
                rp = f"r{rep}_" if reps > 1 else ""
                # attn pairs for o_proj: one tile per seq, [128, 4 jt, 512 t]
                a8_sb = [apool.tile([128, HC, QLEN], F8, tag="a8",
                                    name=f"{rp}a8_{b}", bufs=BUFS["attn"])
                         for b in range(B)]
                ar_sb = [apool.tile([128, HC, QLEN], F8, tag="ar",
                                    name=f"{rp}ar_{b}", bufs=BUFS["ar"])
                         for b in range(B)]

                from contextlib import ExitStack
                with ExitStack() as stk:
                    ep = stk.enter_context
                    cspool = ep(tc.tile_pool(name="cs", bufs=BUFS["cs"]))
                    hidpool = ep(tc.tile_pool(name="hid", bufs=BUFS["hid"]))
                    wqpool = ep(tc.tile_pool(name="wst", bufs=BUFS["wq"]))
                    wvpool = ep(tc.tile_pool(name="wvst", bufs=BUFS["wv"]))
                    qkrpool = ep(tc.tile_pool(name="qkr", bufs=BUFS["qkr"]))
                    rppool = ep(tc.tile_pool(name="rope", bufs=BUFS["qs"]))
                    vpool = ep(tc.tile_pool(name="vsb", bufs=BUFS["vsb"]))
                    khpool = ep(tc.tile_pool(name="khp", bufs=BUFS["kh"]))
                    vhpool = ep(tc.tile_pool(name="vhp", bufs=BUFS["vh"]))
                    epool = ep(tc.tile_pool(name="expp", bufs=BUFS["exp"]))
                    wopool = ep(tc.tile_pool(name="wop", bufs=BUFS["wo"]))
                    stpool = ep(tc.tile_pool(name="stg", bufs=BUFS["stg"]))
                    smpool = ep(tc.tile_pool(name="smol", bufs=BUFS["smol"]))

                    def attn_gen(b, qk_rot, v_sb, vht, kh_ts):
                        """one yield per (h, st) step + one per head tail;
                        emits seq b's attention, pumped between DR bursts
                        of the next seq's projections."""
                        S = SH[b] + 4
                        for h in range(HC):
                            pv = pspool.tile([128, QLEN], F32, tag="ps",
                                             name=f"{rp}pv{b}_{h}")
                            dn = smpool.tile([1, QLEN], F32, tag="dn")
                            for st in range(S):
                                sc = pspool.tile([128, QLEN], F32, tag="ps",
                                                 name=f"{rp}sc{b}_{h}_{st}")
                                if st < SH[b]:
                                    lhsT = kh_ts[h][:, st * 128:(st + 1) * 128]
                                else:
                                    j = st - SH[b]
                                    lhsT = qk_rot[4 + h][:,
                                                         j * 128:(j + 1) * 128]
                                nc.tensor.matmul(sc[:], lhsT, qk_rot[h][:],
                                                 start=True, stop=True)
                                if st == SH[b] - 1 and hist[b] != hv[b]:
                                    nc.vector.tensor_scalar_add(
                                        sc[:], sc[:], pad_t[:, b:b + 1])
                                if st >= SH[b]:
                                    j = st - SH[b]
                                    nc.vector.tensor_add(
                                        sc[:], sc[:],
                                        mask_t[:, 384 - 128 * j:896 - 128 * j])
                                ex = epool.tile([128, QLEN], BF16, tag="exp")
                                nc.scalar.activation(
                                    ex[:], sc[:],
                                    mybir.ActivationFunctionType.Exp,
                                    scale=(exp_hist if st < SH[b]
                                           else exp_new))
                                alr = smpool.tile([128, QLEN], F32, tag="alr",
                                                  bufs=1)
                                nc.gpsimd.partition_all_reduce(
                                    alr[:], ex[:], channels=128,
                                    reduce_op=bass_isa.ReduceOp.add)
                                if st == 0:
                                    nc.vector.tensor_copy(dn[:], alr[0:1, :])
                                else:
                                    nc.vector.tensor_add(dn[:], dn[:],
                                                         alr[0:1, :])
                                yield
                                if st < SH[b]:
                                    vt = vht[:, st, h * 128:(h + 1) * 128]
                                else:
                                    vt = v_sb[st - SH[b]][:,
                                                          h * 128:
                                                          (h + 1) * 128]
                                nc.tensor.matmul(pv[:], vt, ex[:],
                                                 start=(st == 0),
                                                 stop=(st == S - 1))
                            rc = smpool.tile([1, QLEN], F32, tag="rc", bufs=1)
                            nc.vector.reciprocal(rc[:], dn[:])
                            rc2 = smpool.tile([1, QLEN], F32, tag="rc2",
                                              bufs=1)
                            nc.vector.tensor_scalar_mul(rc2[:], rc[:],
                                                        1.0 / S_A)
                            bcs = smpool.tile([128, QLEN], F32, tag="bcs")
                            nc.gpsimd.partition_broadcast(bcs[:], rc2[:])
                            atf = rppool.tile([128, QLEN], F32, tag="atf",
                                              bufs=2)
                            nc.vector.tensor_mul(atf[:], pv[:], bcs[:])
                            nc.scalar.copy(a8_sb[b][:, h, :], atf[:])
                            nc.vector.tensor_sub(ar_sb[b][:, h, :], atf[:],
                                                 a8_sb[b][:, h, :])
                            yield

                    pending = []

                    def pump(n=1):
                        for _ in range(n):
                            while pending:
                                try:
                                    next(pending[0])
                                    break
                                except StopIteration:
                                    pending.pop(0)
                            else:
                                return

                    def attn_dmas(b):
                        """issue kh/vh DMAs for seq b"""
                        vht = None
                        if SH[b]:
                            vht = vhpool.tile([128, SH[b], W], BF16, tag="vh",
                                              name=f"{rp}vh_t{b}")
                            nc.sync.dma_start(
                                vht[:],
                                vh_d[b][:].rearrange("(s p) c -> p s c",
                                                     p=128))
                        kh_ts = []
                        for h in range(HC):
                            if SH[b]:
                                kh_t = khpool.tile([128, hv[b]], BF16,
                                                   tag="kh",
                                                   name=f"{rp}kh{b}_{h}")
                                nc.sync.dma_start(
                                    kh_t[:],
                                    kh_d[b][h * 128:(h + 1) * 128, :])
                                kh_ts.append(kh_t)
                            else:
                                kh_ts.append(None)
                        return vht, kh_ts

                    def start_attn(b, qk_rot, v_sb, dmas=None):
                        vht, kh_ts = dmas if dmas else attn_dmas(b)
                        pending.append(attn_gen(b, qk_rot, v_sb, vht, kh_ts))

                    wo_q = []

                    def wo_dma(ic):
                        isl = slice(ic * 512, (ic + 1) * 512)
                        woa = wopool.tile([128, 4, 512], F8, tag="wo",
                                          name=f"{rp}woa{ic}")
                        nc.sync.dma_start(
                            woa[:],
                            woa_d[:, isl].rearrange("(s p) c -> p s c",
                                                    p=128))
                        wor = wopool.tile([128, 4, 512], F8, tag="wo",
                                          name=f"{rp}wor{ic}")
                        nc.sync.dma_start(
                            wor[:],
                            wor_d[:, isl].rearrange("(s p) c -> p s c",
                                                    p=128))
                        return woa, wor

                    def wq_dma(b, rt):
                        wsl = slice(rt * 32 * 128, (rt + 1) * 32 * 128)
                        wqa = wqpool.tile([128, 32, 128], F8, tag="wq",
                                          name=f"{rp}wqa{b}_{rt}")
                        nc.sync.dma_start(
                            wqa[:], wqa_d[:, wsl]
                            .rearrange("p (s c) -> p s c", c=128))
                        wqrr = wqpool.tile([128, 32, 128], F8, tag="wq",
                                           name=f"{rp}wqr{b}_{rt}")
                        nc.sync.dma_start(
                            wqrr[:], wqr_d[:, wsl]
                            .rearrange("p (s c) -> p s c", c=128))
                        return wqa, wqrr

                    prev = None
                    for b in range(B):
                        tsl = slice(b * QLEN, (b + 1) * QLEN)
                        wq_next = wq_dma(b, 0)
                        # hidden pair for this seq, split in slab halves
                        # so rt0 matmuls start after the first 2048-row DMA
                        hid_t = {}
                        for nmh, src_d in (("ha", ha_d), ("hr", hr_d)):
                            for hh in range(2):
                                ht = hidpool.tile([128, 16, QLEN], F8,
                                                  tag="hid",
                                                  name=f"{rp}{nmh}{b}_{hh}")
                                nc.sync.dma_start(
                                    ht[:],
                                    src_d[hh * 2048:(hh + 1) * 2048, tsl]
                                    .rearrange("(s p) t -> p s t", p=128))
                                hid_t[nmh, hh] = ht
                        cos_t = cspool.tile([128, QLEN], F32, tag="cos",
                                            bufs=2)
                        nc.sync.dma_start(cos_t[:], cos_d[:, tsl])
                        sin_t = cspool.tile([128, QLEN], F32, tag="sin",
                                            bufs=2)
                        nc.sync.dma_start(sin_t[:], sin_d[:, tsl])
                        emit_const_dmas()

                        if prev is not None:
                            start_attn(*prev)

                        # ---- QK projection (3-term residual fp8 DoubleRow)
                        # + RoPE -> qk_rot[rt] = [128 d, 512 t] bf16 (raw)
                        v_sb = [vpool.tile([128, W], BF16, tag="vsb",
                                           name=f"{rp}vsb{b}_{i}")
                                for i in range(4)]
                        v_ps = [pspool.tile([128, W], F32, tag="ps",
                                            name=f"{rp}vps{b}_{i}")
                                for i in range(4)]
                        qk_rot = []
                        for rt in range(8):
                            wqa, wqrr = wq_next
                            if rt < 7:
                                wq_next = wq_dma(b, rt + 1)
                            pq = pspool.tile([128, QLEN], F32, tag="ps",
                                             name=f"{rp}pq{b}_{rt}")
                            n_dr = 0
                            for hh in range(2):
                                for lw, rhn in ((wqa, "ha"), (wqa, "hr"),
                                                (wqrr, "ha")):
                                    rh = hid_t[rhn, hh]
                                    for i in range(8):
                                        n_dr += 1
                                        nc.tensor.matmul(
                                            pq[:],
                                            lw[:, 16 * hh + 2 * i:
                                               16 * hh + 2 * i + 2, :],
                                            rh[:, 2 * i:2 * i + 2, :],
                                            start=(n_dr == 1),
                                            stop=(n_dr == 48),
                                            perf_mode=DR)
                                        if n_dr % 8 == 0:
                                            pump(1)
                            qs = rppool.tile([128, QLEN], F32R, tag="qs")
                            nc.scalar.copy(qs[:], pq[:])
                            rot = pspool.tile([128, QLEN], F32, tag="ps",
                                              name=f"{rp}rot{b}_{rt}")
                            nc.tensor.matmul(rot[:], pt_t[:], qs[:],
                                             start=True, stop=True)
                            t1 = rppool.tile([128, QLEN], F32, tag="t1",
                                             bufs=BUFS["ropet"])
                            nc.vector.tensor_mul(t1[:], rot[:], sin_t[:])
                            t2 = rppool.tile([128, QLEN], F32, tag="t2",
                                             bufs=BUFS["ropet"])
                            nc.vector.tensor_mul(t2[:], qs[:], cos_t[:])
                            qr = qkrpool.tile([128, QLEN], BF16, tag="qkr")
                            nc.vector.tensor_add(qr[:], t1[:], t2[:])
                            qk_rot.append(qr)
                        # V proj: out [128 tok, 512 d] per tok-chunk tt;
                        # wv pair tiles stream once per seq (i outer)
                        for i in range(16):
                            wva = wvpool.tile([128, 2, W], F8, tag="wv",
                                              name=f"{rp}wva{b}_{i}")
                            nc.sync.dma_start(
                                wva[:], wva_d[i * 256:(i + 1) * 256, :]
                                .rearrange("(s p) c -> p s c", p=128))
                            wvrr = wvpool.tile([128, 2, W], F8, tag="wv",
                                               name=f"{rp}wvr{b}_{i}")
                            nc.sync.dma_start(
                                wvrr[:], wvr_d[i * 256:(i + 1) * 256, :]
                                .rearrange("(s p) c -> p s c", p=128))
                            hh, i2 = divmod(i, 8)
                            for tt in range(4):
                                csl = slice(tt * 128, (tt + 1) * 128)
                                for k3, (lwn, rh) in enumerate(
                                        (("ha", wva), ("ha", wvrr),
                                         ("hr", wva))):
                                    lw = hid_t[lwn, hh]
                                    nc.tensor.matmul(
                                        v_ps[tt][:],
                                        lw[:, 2 * i2:2 * i2 + 2, csl],
                                        rh[:], start=(i == 0 and k3 == 0),
                                        stop=(i == 15 and k3 == 2),
                                        perf_mode=DR)
                            pump(1)
                        for tt in range(4):
                            nc.scalar.activation(
                                v_sb[tt][:], v_ps[tt][:],
                                mybir.ActivationFunctionType.Copy,
                                scale=v_copy)
                        prev = (b, qk_rot, v_sb)
                        if b == B - 1:
                            last_dmas = attn_dmas(b)
                            wo_q.append(wo_dma(0))

                    start_attn(*prev, dmas=last_dmas)

                    # ---- o_proj partial (3-term residual fp8 DoubleRow),
                    # interleaved with the last seq's attention
                    wo_q.append(wo_dma(1))

                    def o_unit(ic, tt, woa, wor, do_pump):
                        isl = slice(ic * 512, (ic + 1) * 512)
                        b, q = tt // 4, tt % 4
                        qsl = slice(q * 128, (q + 1) * 128)
                        po = pspool.tile([128, 512], F32, tag="ps",
                                         name=f"{rp}po{ic}_{tt}")
                        n_dr = 0
                        for la, lw in ((a8_sb[b], woa), (a8_sb[b], wor),
                                       (ar_sb[b], woa)):
                            for p2 in range(2):
                                n_dr += 1
                                nc.tensor.matmul(
                                    po[:],
                                    la[:, 2 * p2:2 * p2 + 2, qsl],
                                    lw[:, 2 * p2:2 * p2 + 2, :],
                                    start=(n_dr == 1), stop=(n_dr == 6),
                                    perf_mode=DR)
                        st_ = stpool.tile([128, 512], F16, tag="stg")
                        if tt % 2 == 0:
                            nc.scalar.activation(
                                st_[:], po[:],
                                mybir.ActivationFunctionType.Copy,
                                scale=o_copy)
                        else:
                            nc.vector.tensor_scalar_mul(st_[:], po[:],
                                                        o_copy)
                        nc.sync.dma_start(
                            out_d[tt * 128:(tt + 1) * 128, isl], st_[:])
                        if do_pump:
                            pump(1)

                    # phase 1: seqs 0-2, last seq's attention pumped in slots
                    for ic in range(8):
                        woa, wor = wo_q[ic]
                        for tt in range(12):
                            o_unit(ic, tt, woa, wor, tt % 2 == 1)
                        if ic + 2 < 8:
                            wo_q.append(wo_dma(ic + 2))
                        if ic == 6:
                            wo_q.append(wo_dma(0))  # phase-2 reload
                    pump(100)  # drain remaining attention units
                    # phase 2: seq 3 (attention now complete), wo re-streamed
                    wo_q.append(wo_dma(1))
                    for ic in range(8):
                        woa, wor = wo_q[8 + ic]
                        if ic + 2 < 8:
                            wo_q.append(wo_dma(ic + 2))
                        for tt in range(12, 16):
                            o_unit(ic, tt, woa, wor, False)
    nc.compile()
    return {"nc": nc}


def _get(hist, scales, reps=1):
    key = (hist, scales, reps)
    if key not in _cache:
        _cache[key] = _build(hist, scales, reps)
    return _cache[key]


def _resid8(x, s):
    """x/s -> (a8, r8) fp8 pair, shared scale"""
    import ml_dtypes
    F8 = ml_dtypes.float8_e4m3
    xs = (np.asarray(x, np.float32) / s).astype(np.float32)
    a8 = xs.astype(F8)
    r8 = (xs - a8.astype(np.float32)).astype(F8)
    return a8, r8


def prepare_in_maps(inputs, reps=1):
    import ml_dtypes
    BF16 = ml_dtypes.bfloat16

    hidden = np.asarray(inputs["hidden_states"], np.float32)
    w_pack = np.asarray(inputs["w_pack"], np.float32)
    w_o = np.asarray(inputs["w_o"], np.float32)
    kc = np.asarray(inputs["key_cache"], np.float32).reshape(NBLOCKS * BS, H, D)
    vc = np.asarray(inputs["value_cache"], np.float32).reshape(NBLOCKS * BS, H, D)
    bo = np.asarray(inputs["block_offsets"], np.int32)
    hist = tuple(int(x) for x in np.asarray(inputs["history_lengths"]))
    assert all(0 <= h and h + QLEN <= MAXBLK * BS for h in hist)
    hv = [_round128(h) for h in hist]

    s_h = _pow2_scale(np.abs(hidden).max())
    s_wqk = _pow2_scale(np.abs(w_pack[:2 * HID]).max())
    s_wv = _pow2_scale(np.abs(w_pack[2 * HID:]).max())
    s_wo = _pow2_scale(np.abs(w_o).max())
    scales = (s_h, s_wqk, s_wv, s_wo)

    built = _get(hist, scales, reps)

    hiddenT = np.ascontiguousarray(hidden.T)
    ha, hr = _resid8(hiddenT, s_h)

    in_maps = []
    for c in range(NCORES):
        rs = slice(c * W, (c + 1) * W)
        wqk = np.concatenate(
            [w_pack[rs], w_pack[HID + c * W:HID + (c + 1) * W]], axis=0)
        # wq layout [p, rt, s, m]: wqk[rt*128+m, s*128+p]
        wq_l = np.ascontiguousarray(
            wqk.reshape(8, 128, 32, 128).transpose(3, 0, 2, 1)
            .reshape(128, 8 * 32 * 128))
        wqa, wqr = _resid8(wq_l, s_wqk)
        wv = w_pack[2 * HID + c * W:2 * HID + (c + 1) * W]
        wva, wvr = _resid8(np.ascontiguousarray(wv.T), s_wv)
        woa, wor = _resid8(np.ascontiguousarray(w_o[:, rs].T), s_wo)
        im = {
            "ha": ha, "hr": hr,
            "wqa": wqa, "wqr": wqr,
            "wva": wva, "wvr": wvr,
            "woa": woa, "wor": wor,
        }
        for b in range(B):
            if not hv[b]:
                continue
            nblk = (hist[b] + BS - 1) // BS
            rows = (bo[b, :nblk, None] * BS +
                    np.arange(BS)[None, :]).reshape(-1)[:hist[b]]
            khp = np.zeros((hv[b], HC, D), np.float32)
            khp[:hist[b]] = kc[rows][:, c * HC:(c + 1) * HC, :]
            vhp = np.zeros((hv[b], HC, D), np.float32)
            vhp[:hist[b]] = vc[rows][:, c * HC:(c + 1) * HC, :]
            im[f"khT{b}"] = np.ascontiguousarray(
                khp.transpose(1, 2, 0).reshape(W, hv[b])).astype(BF16)
            im[f"vh{b}"] = np.ascontiguousarray(
                vhp.reshape(hv[b], W)).astype(BF16)
        in_maps.append(im)
    return built["nc"], in_maps


def kernel(**inputs):
    global last_results
    from concourse.bass_utils import run_bass_kernel_spmd

    nc, in_maps = prepare_in_maps(inputs)
    last_results = run_bass_kernel_spmd(nc, in_maps,
                                        core_ids=list(range(NCORES)))
    acc = np.zeros((T, HID), np.float32)
    for c in range(NCORES):
        acc += last_results.results[c]["out"].astype(np.float32)
    return acc
